# revision 7
# baseline (speedup 1.0000x reference)
"""Trainium2 Bass kernel for nn_LongTermMemoryMLP.

Per-batch-weight 3-layer MLP:
    h0 = relu(q @ W0^T + b0); h1 = relu(h0 @ W1^T + b1); out = h1 @ W2^T + b2
with q: [B,S,DIN], W0: [B,DH,DIN], W1: [B,DH,DH], W2: [B,DOUT,DH], B=8.

Sharding: data-parallel over batch — one batch sample (and its weight slabs)
per NeuronCore, 8 cores, no cross-core communication.

Device-side strategy: activations are kept feature-major ([feature, seq],
feature on partitions) so every layer is a plain accumulated matmul with the
(pre-transposed) weights as the stationary operand and the activations as the
moving operand — no on-chip transposes. The final layer flips orientation
(stationary = activation tile, moving = W2^T) so the output lands seq-major
and can be DMA'd out contiguously. Inputs are pre-transposed and cast to
bf16 on the host (weights/activations bf16, accumulation fp32 in PSUM).
"""

import numpy as np

import ml_dtypes

import concourse.bass as bass
import concourse.tile as tile
from concourse import bacc, mybir
from concourse.bass_utils import run_bass_kernel_spmd

B, S, DIN, DH, DOUT = 8, 4096, 512, 1024, 512
SC = 512  # seq chunk processed per pipeline iteration

BF16 = mybir.dt.bfloat16
F32 = mybir.dt.float32


def build_nc():
    nc = bacc.Bacc("TRN2")
    qT = nc.dram_tensor("qT", (DIN, S), BF16, kind="ExternalInput")
    w0t = nc.dram_tensor("w0t", (DIN, DH), BF16, kind="ExternalInput")
    w1t = nc.dram_tensor("w1t", (DH, DH), BF16, kind="ExternalInput")
    w2t = nc.dram_tensor("w2t", (DH, DOUT), BF16, kind="ExternalInput")
    b0 = nc.dram_tensor("b0", (DH,), F32, kind="ExternalInput")
    b1 = nc.dram_tensor("b1", (DH,), F32, kind="ExternalInput")
    b2 = nc.dram_tensor("b2", (DOUT,), F32, kind="ExternalInput")
    out = nc.dram_tensor("out", (S, DOUT), F32, kind="ExternalOutput")

    K0 = DIN // 128   # 4  k-tiles, layer 0
    K1 = DH // 128    # 8  k-tiles, layers 1/2
    M0 = DH // 128    # 8  m-tiles (feature tiles of h0/h1)
    MT = SC // 128    # 4  seq m-tiles per chunk, layer 2
    NCH = S // SC     # 8  chunks

    Relu = mybir.ActivationFunctionType.Relu

    with tile.TileContext(nc) as tc:
        with (
            tc.tile_pool(name="weights", bufs=1) as wpool,
            tc.tile_pool(name="biases", bufs=1) as bpool,
            tc.tile_pool(name="acts", bufs=2) as apool,
            tc.tile_pool(name="qin", bufs=2) as qpool,
            tc.tile_pool(name="outp", bufs=4) as opool,
            tc.tile_pool(name="psum0", bufs=2, space="PSUM") as ppool0,
            tc.tile_pool(name="psum1", bufs=3, space="PSUM") as ppool1,
            tc.tile_pool(name="psum2", bufs=3, space="PSUM") as ppool2,
        ):
            # Pre-warm the PE clock gate (HAM) with dummy matmuls on garbage
            # data while the startup DMAs land: the real matmul stream then
            # runs at 2.4 GHz from its first instruction.
            g_lhs = apool.tile([128, 128], BF16, tag="warm_lhs")
            g_rhs = apool.tile([128, SC], BF16, tag="warm_rhs")
            nc.vector.memset(g_lhs, 0.0)
            nc.vector.memset(g_rhs, 0.0)
            warm_ps = ppool0.tile([128, SC], F32, tag="ps0")
            N_WARM = 16
            for i in range(N_WARM):
                nc.tensor.matmul(
                    warm_ps, lhsT=g_lhs, rhs=g_rhs,
                    start=(i == 0), stop=(i == N_WARM - 1),
                )

            # Startup-critical loads first: layer-0 weights + chunk-0 inputs,
            # split into column slices so they spread across DMA queues
            # (per-queue bandwidth is only ~25 GB/s) and the matmul stream
            # starts as soon as possible. W1/W2 stream in behind them,
            # hidden under chunk-0 layer-0 compute.
            w0_sb = [wpool.tile([128, DH], BF16, tag=f"w0_{k}", name=f"w0_{k}") for k in range(K0)]
            q0_sb = [qpool.tile([128, SC], BF16, tag=f"q_{k}", name=f"q0_{k}") for k in range(K0)]
            for k in range(K0):
                for j in range(4):
                    nc.sync.dma_start(
                        out=w0_sb[k][:, j * 256:(j + 1) * 256],
                        in_=w0t[k * 128:(k + 1) * 128, j * 256:(j + 1) * 256],
                    )
                for j in range(2):
                    nc.sync.dma_start(
                        out=q0_sb[k][:, j * 256:(j + 1) * 256],
                        in_=qT[k * 128:(k + 1) * 128, j * 256:(j + 1) * 256],
                    )
            b0_sb = bpool.tile([128, M0], F32, tag="b0")
            nc.gpsimd.dma_start(out=b0_sb, in_=b0[:].rearrange("(m p) -> p m", p=128))

            w1_sb = []
            for k in range(K1):
                t = wpool.tile([128, DH], BF16, tag=f"w1_{k}")
                nc.sync.dma_start(out=t, in_=w1t[k * 128:(k + 1) * 128, :])
                w1_sb.append(t)
            b1_sb = bpool.tile([128, M0], F32, tag="b1")
            nc.gpsimd.dma_start(out=b1_sb, in_=b1[:].rearrange("(m p) -> p m", p=128))

            w2_sb = []
            for k in range(K1):
                t = wpool.tile([128, DOUT], BF16, tag=f"w2_{k}")
                nc.sync.dma_start(out=t, in_=w2t[k * 128:(k + 1) * 128, :])
                w2_sb.append(t)
            b2_sb = bpool.tile([128, DOUT], F32, tag="b2")
            b2_ap = b2[:]
            b2_bcast = bass.AP(
                tensor=b2_ap.tensor,
                offset=b2_ap.offset,
                ap=[[0, 128]] + [list(d) for d in b2_ap.ap],
            )
            nc.gpsimd.dma_start(out=b2_sb, in_=b2_bcast)

            for c in range(NCH):
                s0 = c * SC
                if c == 0:
                    q_sb = q0_sb
                else:
                    q_sb = []
                    for k in range(K0):
                        t = qpool.tile([128, SC], BF16, tag=f"q_{k}")
                        nc.sync.dma_start(
                            out=t, in_=qT[k * 128:(k + 1) * 128, s0:s0 + SC]
                        )
                        q_sb.append(t)

                h0_sb = []
                for m in range(M0):
                    ps = ppool0.tile([128, SC], F32, tag="ps0")
                    for k in range(K0):
                        nc.tensor.matmul(
                            ps,
                            lhsT=w0_sb[k][:, m * 128:(m + 1) * 128],
                            rhs=q_sb[k],
                            start=(k == 0),
                            stop=(k == K0 - 1),
                        )
                    h = apool.tile([128, SC], BF16, tag=f"h0_{m}")
                    nc.scalar.activation(h, ps, Relu, bias=b0_sb[:, m:m + 1])
                    h0_sb.append(h)

                h1_sb = []
                for m in range(M0):
                    ps = ppool1.tile([128, SC], F32, tag="ps1")
                    for k in range(K1):
                        nc.tensor.matmul(
                            ps,
                            lhsT=w1_sb[k][:, m * 128:(m + 1) * 128],
                            rhs=h0_sb[k],
                            start=(k == 0),
                            stop=(k == K1 - 1),
                        )
                    h = apool.tile([128, SC], BF16, tag=f"h1_{m}")
                    nc.scalar.activation(h, ps, Relu, bias=b1_sb[:, m:m + 1])
                    h1_sb.append(h)

                for mt in range(MT):
                    ps = ppool2.tile([128, DOUT], F32, tag="ps2")
                    for k in range(K1):
                        nc.tensor.matmul(
                            ps,
                            lhsT=h1_sb[k][:, mt * 128:(mt + 1) * 128],
                            rhs=w2_sb[k],
                            start=(k == 0),
                            stop=(k == K1 - 1),
                        )
                    ot = opool.tile([128, DOUT], F32, tag="ot")
                    nc.vector.tensor_add(ot, ps, b2_sb)
                    nc.sync.dma_start(
                        out=out[s0 + mt * 128:s0 + (mt + 1) * 128, :], in_=ot
                    )
    nc.finalize()
    return nc


_NC = None


def _get_nc():
    global _NC
    if _NC is None:
        _NC = build_nc()
    return _NC


def make_in_maps(inputs):
    bf16 = ml_dtypes.bfloat16
    q, W0, b0, W1, b1, W2, b2 = (
        inputs["query"], inputs["W0"], inputs["b0"], inputs["W1"],
        inputs["b1"], inputs["W2"], inputs["b2"],
    )
    in_maps = []
    for b in range(B):
        in_maps.append({
            "qT": np.ascontiguousarray(np.asarray(q[b]).T).astype(bf16),
            "w0t": np.ascontiguousarray(np.asarray(W0[b]).T).astype(bf16),
            "w1t": np.ascontiguousarray(np.asarray(W1[b]).T).astype(bf16),
            "w2t": np.ascontiguousarray(np.asarray(W2[b]).T).astype(bf16),
            "b0": np.asarray(b0[b], dtype=np.float32),
            "b1": np.asarray(b1[b], dtype=np.float32),
            "b2": np.asarray(b2[b], dtype=np.float32),
        })
    return in_maps


def run(inputs, trace=False):
    nc = _get_nc()
    in_maps = make_in_maps(inputs)
    res = run_bass_kernel_spmd(nc, in_maps, core_ids=list(range(B)), trace=trace)
    out = np.stack([np.asarray(r["out"], dtype=np.float32) for r in res.results])
    return out, res


def kernel(**inputs) -> np.ndarray:
    out, _ = run(inputs, trace=False)
    return out
